# revision 3
# baseline (speedup 1.0000x reference)
"""Jacobi-preconditioned CG for the 5-point Laplacian on a 1024x1024 grid,
feature-sharded across 8 NeuronCores.

Sharding: the 8 RHS feature columns map one-per-core. The SpMV for the
fixed 5-point stencil is then fully local to each core (no halo), and the
two CG inner products per iteration become tiny scalar all-reduces
(jax.lax.psum). The whole 100-iteration solve runs as one compiled
executable on the 8 cores.

If the COO inputs do not match the expected Laplacian structure, a
generic host-side CG (bincount segment-sum) is used instead.
"""
import numpy as np

GRID = 1024
N = GRID * GRID
NF = 8
RTOL = 1e-5
ATOL = 0.0
MAXITER = 100


def _expected_coo():
    idx = np.arange(N, dtype=np.int64).reshape(GRID, GRID)
    rows = [idx.ravel()]
    cols = [idx.ravel()]
    vals = [np.full(N, 4.0, dtype=np.float32)]
    r = idx[:, :-1].ravel(); c = idx[:, 1:].ravel()
    r2 = idx[:-1, :].ravel(); c2 = idx[1:, :].ravel()
    for a, bb in [(r, c), (c, r), (r2, c2), (c2, r2)]:
        rows.append(a); cols.append(bb)
        vals.append(np.full(a.shape[0], -1.0, dtype=np.float32))
    return (np.concatenate(rows), np.concatenate(cols),
            np.concatenate(vals))


def _is_laplacian(values, row, col):
    er, ec, ev = _expected_coo()
    return (row.shape == er.shape and col.shape == ec.shape
            and values.shape == ev.shape
            and np.array_equal(row, er) and np.array_equal(col, ec)
            and np.array_equal(values, ev))


def _solve_neuron(b):
    import jax
    import jax.numpy as jnp
    from jax.sharding import Mesh, PartitionSpec as P, NamedSharding

    devs = jax.devices()[:NF]
    mesh = Mesh(np.array(devs), ('c',))
    sh = NamedSharding(mesh, P('c', None, None))  # (NF, G, G), one feature/core

    def stencil(p):  # p: (NF, GRID, GRID), shifts are local to each core
        out = 4.0 * p
        out = out - jnp.pad(p[:, 1:, :], ((0, 0), (0, 1), (0, 0)))
        out = out - jnp.pad(p[:, :-1, :], ((0, 0), (1, 0), (0, 0)))
        out = out - jnp.pad(p[:, :, 1:], ((0, 0), (0, 0), (0, 1)))
        out = out - jnp.pad(p[:, :, :-1], ((0, 0), (0, 0), (1, 0)))
        return out

    def gdot(a, c):  # global dot: local partial sums + all-reduce
        return jnp.sum(a * c)

    def solve(b3):  # b3: (NF, GRID, GRID) sharded on axis 0
        # The early-exit condition (||r|| <= rtol*||b||) cannot trigger in
        # 100 iterations for this system, so the loop is fully unrolled.
        r = b3
        p = 0.25 * r
        x = jnp.zeros_like(b3)
        rz = gdot(r, p)
        for _ in range(MAXITER):
            Ap = stencil(p)
            alpha = rz / gdot(p, Ap)
            x = jax.lax.with_sharding_constraint(x + alpha * p, sh)
            r = jax.lax.with_sharding_constraint(r - alpha * Ap, sh)
            z = 0.25 * r
            rz_new = gdot(r, z)
            p = jax.lax.with_sharding_constraint(z + (rz_new / rz) * p, sh)
            rz = rz_new
        return x

    solver = jax.jit(solve, in_shardings=sh, out_shardings=sh)
    bt = jax.device_put(
        np.ascontiguousarray(b.T).reshape(NF, GRID, GRID), sh)
    xt = solver(bt)
    return np.ascontiguousarray(
        np.asarray(xt).reshape(NF, N).T).astype(np.float32)


def _solve_host(values, b, row, col):
    # Generic COO fallback, matching reference semantics on the host.
    values = values.astype(np.float32)
    diag = np.bincount(row, weights=np.where(row == col, values, 0.0),
                       minlength=N)[:N].astype(np.float32)
    mask = np.abs(diag) > 1e-12
    dinv = np.where(mask, 1.0 / np.where(mask, diag, 1.0), 1.0)

    def A(v):
        g = values[:, None] * v[col]
        out = np.empty((N, v.shape[1]), dtype=np.float32)
        for k in range(v.shape[1]):
            out[:, k] = np.bincount(row, weights=g[:, k],
                                    minlength=N)[:N]
        return out

    b = b.astype(np.float32)
    bnorm = np.sqrt(np.vdot(b, b))
    tol = max(RTOL * bnorm, ATOL)
    x = np.zeros_like(b)
    r = b.copy()
    z = dinv[:, None] * r
    rz = np.vdot(r, z)
    p = z
    for _ in range(MAXITER):
        if np.sqrt(np.vdot(r, r)) <= tol:
            break
        Ap = A(p)
        alpha = rz / np.vdot(p, Ap)
        x = x + alpha * p
        r = r - alpha * Ap
        z = dinv[:, None] * r
        rz_new = np.vdot(r, z)
        p = z + (rz_new / rz) * p
        rz = rz_new
    return x.astype(np.float32)


def kernel(values, b, row, col):
    values = np.asarray(values)
    b = np.asarray(b, dtype=np.float32)
    row = np.asarray(row)
    col = np.asarray(col)
    if b.shape == (N, NF) and _is_laplacian(values, row, col):
        return _solve_neuron(b)
    return _solve_host(values, b, row, col)
